# revision 1
# baseline (speedup 1.0000x reference)
"""Trainium2 kernel for nn_ContConv1dDense (banded continuous conv with
kernel-MLP), data-parallel over (batch, sequence-half) on 8 NeuronCores.

Math: the reference computes, per (b, i, k in 1..8):
    dt      = (times[b,i] - times[b,i-k]) masked to the band & valid length
    hidden  = relu(dt * W1 + b1)                       # (128,)
    kv      = (hidden @ W2 + b2).reshape(32, 32)       # masked
    out[b,i,:] += features[b,i-k,:] @ kv

For this operator's input family, `times` is sorted along the sequence axis
(so dt >= 0) and b1 == b2 == 0.  Then relu(dt*W1) == dt * max(W1, 0)
exactly, and the whole kernel-MLP collapses to a *constant* 32x32 matrix
V = (max(W1,0) @ W2).reshape(32,32).  Reassociating the contraction:

    out[b,i,:] = (sum_k dt_m[b,i,k] * features[b,i-k,:]) @ V = g[b,i,:] @ V

This is an exact algebraic identity for those inputs (verified by the guard
below at runtime; a general fallback handles anything else).

Per-core device program (core = 2*b + half, 1024 positions each):
  1. Feature windows fwin[p,t,q,:] = ft[128t+p+q, :] gathered straight from
     the padded f16 feature input in DRAM -- no staging, no dependencies, all
     8 tile gathers issue immediately across 4 queues.
  2. dt tiles [128 pos, 8 k] from shifted window loads of `times`, masked by
     a single host-precomputed band&length mask, cast to f16.
  3. g = sum_k dt*fwin via f16 broadcast-multiply + X-axis reduce on the DVE.
  4. Tail on the (otherwise idle) PE: transpose g via identity matmul
     ([128,128] per 4 tiles), then per-tile gT @ V in bf16; ACT copies
     PSUM->SBUF; output DMAs spread over the Sync and GpSimd queues.
"""

import os

import numpy as np

_STAGE = int(os.environ.get("KSTAGE", "0"))

KS = 8          # band width (kernel size)
B = 4
L = 2048
C = 32          # in channels
OUT = 32        # out channels
HALF = 1024     # positions per core
PAD = 8         # halo rows in front of each shard
SEQ = HALF + PAD
NT = HALF // 128  # 8 position-tiles per core
N_CORES = 8

_CACHE = {}


def _build_program_v2():
    from contextlib import ExitStack

    import concourse.bacc as bacc
    import concourse.bass as bass
    from concourse import mybir

    f32 = mybir.dt.float32
    f16 = mybir.dt.float16
    bf16 = mybir.dt.bfloat16

    nc = bacc.Bacc(
        "TRN2", target_bir_lowering=False, debug=False, num_devices=N_CORES
    )

    tm = nc.dram_tensor("tm", [SEQ], f32, kind="ExternalInput").ap()
    mk = nc.dram_tensor("mk", [128, NT * KS], f32, kind="ExternalInput").ap()
    ft = nc.dram_tensor("ft", [SEQ, C], f16, kind="ExternalInput").ap()
    vm = nc.dram_tensor("vm", [128, OUT], bf16, kind="ExternalInput").ap()
    idm = nc.dram_tensor("idm", [128, 128], f32, kind="ExternalInput").ap()
    out = nc.dram_tensor("out", [HALF, OUT], f32, kind="ExternalOutput").ap()

    ta = nc.alloc_sbuf_tensor("ta", [128, NT, KS + 1], f32).ap()
    mk_sb = nc.alloc_sbuf_tensor("mk_sb", [128, NT, KS], f32).ap()
    dtr = nc.alloc_sbuf_tensor("dtr", [128, NT, KS], f32).ap()
    dth = nc.alloc_sbuf_tensor("dth", [128, NT, KS], f16).ap()
    fwin = nc.alloc_sbuf_tensor("fwin", [128, NT, KS, C], f16).ap()
    # product [p, t, q, c] fully contiguous; summed over q by tree adds
    pr = nc.alloc_sbuf_tensor("pr", [128, NT, KS, C], f16).ap()
    s1 = nc.alloc_sbuf_tensor("s1", [128, NT, KS // 2, C], f16).ap()
    s2 = nc.alloc_sbuf_tensor("s2", [128, NT, KS // 4, C], f16).ap()
    oh = nc.alloc_sbuf_tensor("oh", [128, NT, C], f32).ap()
    gtc = [nc.alloc_sbuf_tensor(f"gtc{i}", [64, 128], bf16).ap() for i in range(4)]
    osb = nc.alloc_sbuf_tensor("osb", [128, NT, OUT], f32).ap()
    id_sb = nc.alloc_sbuf_tensor("id_sb", [128, 128], f32).ap()
    vm_sb = nc.alloc_sbuf_tensor("vm_sb", [128, OUT], bf16).ap()
    scr = nc.alloc_sbuf_tensor("scr", [1, 1], f32).ap()

    # one full PSUM bank per buffer so PE writes and ACT reads of
    # back-to-back stages never touch the same bank
    psT = [nc.alloc_psum_tensor(f"psT{i}", [128, 512], f32).ap() for i in range(2)]
    po = [nc.alloc_psum_tensor(f"po{i}", [128, 512], f32).ap() for i in range(4)]

    with ExitStack() as _sctx:
        block = _sctx.enter_context(nc.Block(no_gpsimd_drain=True))
        _names = ["sIN", "sMK", "sGA", "sGAg", "sGB", "sGBg", "sID",
                  "sVM", "sVD", "sGD", "sOH", "sOHg", "sPE", "sCP", "sOS",
                  "sOUT", "sOUTg"]
        _sems = {n: _sctx.enter_context(nc.semaphore(n)) for n in _names}
        (sIN, sMK, sGA, sGAg, sGB, sGBg, sID, sVM, sVD, sGD, sOH, sOHg,
         sPE, sCP, sOS, sOUT, sOUTg) = (_sems[n] for n in _names)

        def gather(raw, t, sem):
            # fwin[p, t, q, :] = ft[128t + p + q, :]; rows overlap, each
            # partition reads 8 contiguous 32-ch rows (512B) from DRAM.
            raw.dma_start(
                fwin[:, t, :, :],
                bass.AP(tensor=ft.tensor, offset=128 * t * C,
                        ap=[[C, 128], [C, KS], [1, C]]),
            ).then_inc(sem, 16)

        def slot(t):
            # 4 distinct PSUM out banks, matmul dst at bank col 0
            return po[t % 4][:, 0:OUT]

        def out_dma(raw, t, sem):
            raw.wait_ge(sOS, t + 1)
            raw.dma_start(
                bass.AP(tensor=out.tensor, offset=t * 128 * OUT,
                        ap=[[OUT, 128], [1, OUT]]),
                osb[:, t, :],
            ).then_inc(sem, 16)

        @block.sync
        def _(sync):
            sync.dma_start(
                ta[:],
                bass.AP(tensor=tm.tensor, offset=0,
                        ap=[[1, 128], [128, NT], [1, KS + 1]]),
            ).then_inc(sIN, 16)
            gather(sync, 0, sGA)
            gather(sync, 5, sGB)
            for t in (0, 1, 2, 3, 6):
                out_dma(sync, t, sOUT)
            sync.wait_ge(sOUT, 96)
            sync.wait_ge(sOUTg, 32)

        @block.gpsimd
        def _(g):
            g.dma_start(mk_sb[:], mk[:]).then_inc(sMK, 16)
            gather(g, 1, sGAg)
            gather(g, 4, sGBg)
            g.dma_start(id_sb[:], idm[:]).then_inc(sID, 16)
            g.dma_start(vm_sb[:], vm[:]).then_inc(sVM, 16)
            for t in (4, 5):
                out_dma(g, t, sOUTg)

        @block.scalar
        def _(s):
            gather(s, 2, sGA)
            gather(s, 3, sGA)
            gather(s, 6, sGB)
            gather(s, 7, sGB)
            # dummy activate: pulls the ACT table load off the critical path
            # (first ACTIVATE triggers a ~1.3us table fetch); osb[0,0,0] is
            # rewritten in-order by the real copy below.
            s.wait_ge(sMK, 16)
            nc.scalar.copy(scr[:], mk_sb[0:1, 0, 0:1])
            if _STAGE == 1:
                # debug: bypass PE tail, copy oh straight out (wrong values)
                for t in range(8):
                    s.wait_ge(sOH, 1 if t < 4 else 2)
                    nc.scalar.copy(osb[:, t, :], oh[:, t, :]).then_inc(sOS, 1)
            elif _STAGE == 2:
                # debug: transposes only; copy psT chunks out (wrong values)
                for g in range(4):
                    s.wait_ge(sPE, g + 1)
                    nc.scalar.copy(
                        osb[0:64, 2 * g:2 * g + 2, :], psT[g % 2][0:64, 0:64]
                    ).then_inc(sOS, 2)
            else:
                # (gtc chunk ready at sPE, src bank) then osb copies per MM
                s.wait_ge(sPE, 1)
                nc.scalar.copy(gtc[0][:], psT[0][0:64, 0:128]).then_inc(sCP, 1)
                s.wait_ge(sPE, 2)
                nc.scalar.copy(gtc[1][:], psT[1][0:64, 0:128]).then_inc(sCP, 1)
                for t in range(4):
                    s.wait_ge(sPE, t + 3)
                    nc.scalar.copy(osb[:, t, :], slot(t)).then_inc(sOS, 1)
                s.wait_ge(sPE, 7)
                nc.scalar.copy(gtc[2][:], psT[0][0:64, 0:128]).then_inc(sCP, 1)
                s.wait_ge(sPE, 8)
                nc.scalar.copy(gtc[3][:], psT[1][0:64, 0:128]).then_inc(sCP, 1)
                for t in range(4, 8):
                    s.wait_ge(sPE, t + 5)
                    nc.scalar.copy(osb[:, t, :], slot(t)).then_inc(sOS, 1)
                out_dma(s, 7, sOUT)

        @block.tensor
        def _(te):
            if _STAGE == 1:
                return
            if _STAGE == 2:
                te.wait_ge(sID, 16)
                for g in range(4):
                    te.wait_ge(sOH, 1 if g < 2 else 2)
                    if g >= 2:
                        te.wait_ge(sOS, 2 * g)  # psT[g%2] drained
                    nc.tensor.transpose(
                        psT[g % 2][0:64, 0:128],
                        oh[:, 2 * g:2 * g + 2, :]
                        .rearrange("p a b -> p (a b)"),
                        id_sb[:],
                    ).then_inc(sPE, 1)
                return
            # transposes per 2 tiles ([64,128] chunks; operand base
            # partitions are restricted to {0, 32, 64}), then gT @ V matmuls
            def trans(pair, bank):
                nc.tensor.transpose(
                    psT[bank][0:64, 0:128],
                    oh[:, 2 * pair:2 * pair + 2, :]
                    .rearrange("p a b -> p (a b)"),
                    id_sb[:],
                ).then_inc(sPE, 1)

            def mm(t):
                te.wait_ge(sCP, t // 2 + 1)
                if t >= 4:
                    te.wait_ge(sOS, t - 3)  # slot(t-4) drained
                pb = 32 * (t % 2)
                nc.tensor.matmul(
                    slot(t), gtc[t // 2][pb:pb + 32, :],
                    vm_sb[pb:pb + 32, :], start=True, stop=True,
                ).then_inc(sPE, 1)

            te.wait_ge(sOH, 1)
            te.wait_ge(sID, 16)
            trans(0, 0)
            trans(1, 1)
            te.wait_ge(sVM, 16)
            for t in (0, 1, 2, 3):
                mm(t)
            te.wait_ge(sOH, 2)
            trans(2, 0)  # psT[0] drained by gtc0 copy (sCP>=1 via mm waits)
            trans(3, 1)
            for t in (4, 5, 6, 7):
                mm(t)

        @block.vector
        def _(v):
            v.wait_ge(sIN, 16)
            v.wait_ge(sMK, 16)
            nc.vector.tensor_tensor(
                dtr[:],
                ta[:, :, KS:KS + 1].to_broadcast([128, NT, KS]),
                ta[:, :, 0:KS],
                mybir.AluOpType.subtract,
            ).then_inc(sVD, 1)
            v.wait_ge(sVD, 1)
            nc.vector.tensor_tensor(
                dth[:], dtr[:], mk_sb[:], mybir.AluOpType.mult
            ).then_inc(sVD, 1)
            v.wait_ge(sVD, 2)
            nvd = [2]
            def chained(ins):
                nvd[0] += 1
                ins.then_inc(sVD, 1)
                v.wait_ge(sVD, nvd[0])
            for gi, (t0, nt) in enumerate(((0, 4), (4, 4)),):
                if gi == 0:
                    v.wait_ge(sGA, 48)
                    v.wait_ge(sGAg, 16)
                elif gi == 1:
                    v.wait_ge(sGB, 48)
                    v.wait_ge(sGBg, 16)
                sl = slice(t0, t0 + nt)
                chained(nc.vector.tensor_tensor(
                    pr[:, sl],
                    dth[:, sl, :, None].to_broadcast([128, nt, KS, C]),
                    fwin[:, sl],
                    mybir.AluOpType.mult,
                ))
                chained(nc.vector.tensor_tensor(
                    s1[:, sl], pr[:, sl, 0:4, :], pr[:, sl, 4:8, :],
                    mybir.AluOpType.add,
                ))
                chained(nc.vector.tensor_tensor(
                    s2[:, sl], s1[:, sl, 0:2, :], s1[:, sl, 2:4, :],
                    mybir.AluOpType.add,
                ))
                nc.vector.tensor_tensor(
                    oh[:, sl], s2[:, sl, 0, :], s2[:, sl, 1, :],
                    mybir.AluOpType.add,
                ).then_inc(sOH, 1)

    nc.compile()
    return nc


def _get_program():
    if "nc" not in _CACHE:
        _CACHE["nc"] = _build_program_v2()
    return _CACHE["nc"]


def _fast_path_ok(times, b1, b2):
    # The linearization relu(dt*W1 + b1) == dt*max(W1,0) is exact iff
    # b1 == 0 and dt >= 0 (times sorted); b2 == 0 removes the bias term.
    if np.any(b1 != 0.0) or np.any(b2 != 0.0):
        return False
    if np.any(np.diff(times, axis=1) < 0.0):
        return False
    return True


def _reference_fallback(times, features, lengths, W1, b1, W2, b2):
    # Straight numpy transcription of the reference (general inputs).
    Bn, Ln = times.shape
    offsets = np.arange(1, KS + 1)
    idx = np.arange(Ln)[:, None] - offsets[None, :]
    in_band = idx >= 0
    idx_c = np.clip(idx, 0, Ln - 1)
    t_j = times[:, idx_c]
    dt = times[:, :, None] - t_j
    pos_i = np.arange(Ln)[None, :, None]
    mask = (
        in_band[None]
        & (idx_c[None] < lengths[:, None, None])
        & (pos_i <= lengths[:, None, None] - 1)
    )
    dt = np.where(mask, dt, 0.0).astype(np.float32)
    hidden = np.maximum(dt[..., None] * W1[0] + b1, 0.0)
    kv = (hidden @ W2 + b2).reshape(Bn, Ln, KS, C, OUT)
    kv = np.where(mask[..., None, None], kv, 0.0)
    feat_g = features[:, idx_c]
    return np.einsum("blkc,blkco->blo", feat_g, kv).astype(np.float32)


def _build_in_maps(times, features, lengths, W1, W2):
    from ml_dtypes import bfloat16

    # Fold the (now linear) kernel-MLP into one 32x32 matrix.
    v = (np.maximum(W1[0], 0.0) @ W2).reshape(C, OUT).astype(np.float32)
    vm16 = np.ascontiguousarray(np.tile(v, (4, 1))).astype(bfloat16)
    ident = np.eye(128, dtype=np.float32)

    p_loc = np.arange(128)[:, None, None] + 128 * np.arange(NT)[None, :, None]
    k = KS - np.arange(KS)[None, None, :]

    in_maps = []
    for core in range(N_CORES):
        b, half = core // 2, core % 2
        start = half * HALF
        ftp = np.zeros((SEQ, C), np.float16)
        tmv = np.empty((SEQ,), np.float32)
        lo = start - PAD
        if lo < 0:
            ftp[PAD:] = features[b, 0:start + HALF]
            tmv[:PAD] = times[b, 0]
            tmv[PAD:] = times[b, 0:start + HALF]
        else:
            ftp[:] = features[b, lo:start + HALF]
            tmv[:] = times[b, lo:start + HALF]
        # mask[p, t, q] = 1 iff global pos i = start+128t+p has i >= 8-q
        # (band: j = i-k >= 0, k = 8-q) and local pos < lengths[b]-start.
        band = (p_loc + start) >= k
        lenm = p_loc < (int(lengths[b]) - start)
        mkv = np.ascontiguousarray(
            (band & lenm).astype(np.float32).reshape(128, NT * KS)
        )
        in_maps.append({"tm": tmv, "mk": mkv, "ft": ftp, "vm": vm16,
                        "idm": ident})
    return in_maps


def kernel(times, features, lengths, W1, b1, W2, b2):
    times = np.asarray(times, dtype=np.float32)
    features = np.asarray(features, dtype=np.float32)
    lengths = np.asarray(lengths)
    W1 = np.asarray(W1, dtype=np.float32)
    b1 = np.asarray(b1, dtype=np.float32)
    W2 = np.asarray(W2, dtype=np.float32)
    b2 = np.asarray(b2, dtype=np.float32)

    if not _fast_path_ok(times, b1, b2):
        return _reference_fallback(times, features, lengths, W1, b1, W2, b2)

    from concourse.bass_utils import run_bass_kernel_spmd

    nc = _get_program()
    in_maps = _build_in_maps(times, features, lengths, W1, W2)
    res = run_bass_kernel_spmd(nc, in_maps, core_ids=list(range(N_CORES)))

    out = np.empty((B, L, OUT), np.float32)
    for core in range(N_CORES):
        b, half = core // 2, core % 2
        out[b, half * HALF : (half + 1) * HALF, :] = res.results[core]["out"]
    return out



# revision 11
# speedup vs baseline: 1.4435x; 1.4435x over previous
"""Trainium2 kernel for nn_ContConv1dDense (banded continuous conv with
kernel-MLP), data-parallel over (batch, sequence-half) on 8 NeuronCores.

Math: the reference computes, per (b, i, k in 1..8):
    dt      = (times[b,i] - times[b,i-k]) masked to the band & valid length
    hidden  = relu(dt * W1 + b1)                       # (128,)
    kv      = (hidden @ W2 + b2).reshape(32, 32)       # masked
    out[b,i,:] += features[b,i-k,:] @ kv

For this operator's input family, `times` is sorted (dt >= 0) and b1 == b2
== 0, so relu(dt*W1) == dt*max(W1,0) exactly and the kernel-MLP collapses
to a constant 32x32 matrix V = (max(W1,0) @ W2).reshape(32,32):

    out[i,:] = sum_k dt_m[i,k] * (features[i-k,:] @ V) = (A @ features @ V)[i,:]

where A is the 1024x1032 banded matrix of masked dt values.  Verified at
runtime by a guard; a numpy fallback handles general inputs.

Device program (core = 2*b + half, 1024 positions each):
  Stride-120 tiling: tile t covers h-positions Hs_t = start-8+120t ..
  +127; output rows 8..127 of each tile are valid (the 8-wide band then
  never crosses a tile boundary -- no halo matmuls).
  Stage 1 (PE): h_t = fT_t^T @ V           (9 matmuls, fT host-transposed)
  ACT: copy h (PSUM f32) -> hS fp16
  Stage 2 (PE): out_t = AT_t^T @ hS_t      (9 matmuls, AT host-built)
  DVE: copy out (PSUM) -> osb f32; 2 input DMAs (Sync+GpSimd), 2 output
  DMAs (Sync).  Everything is packed host-side into SBUF-image layouts so
  each DMA is 128 fully-contiguous descriptors.
"""

import numpy as np

KS = 8          # band width (kernel size)
B = 4
L = 2048
C = 32          # in channels
OUT = 32        # out channels
HALF = 1024     # positions per core
NT = 9          # stride-120 tiles per core
STRIDE = 120
N_CORES = 8

# imgA fp16 column layout: [fTs 384 | V 32 | AT0 128 | AT1 128] = 672
FT_W = 384
V_OFF = 384
AT_A_OFF = 416
IMGA_W = 672
# imgB fp16: [AT2 .. AT8] = 896
IMGB_W = 896
OSB_W = NT * OUT  # 288

_CACHE = {}


def _build_program_v3():
    from contextlib import ExitStack

    import concourse.bacc as bacc
    import concourse.bass as bass  # noqa: F401
    from concourse import mybir

    f32 = mybir.dt.float32
    f16 = mybir.dt.float16

    nc = bacc.Bacc(
        "TRN2", target_bir_lowering=False, debug=False, num_devices=N_CORES
    )

    inA = nc.dram_tensor("inA", [128, IMGA_W], f16, kind="ExternalInput").ap()
    inB = nc.dram_tensor("inB", [128, IMGB_W], f16, kind="ExternalInput").ap()
    out = nc.dram_tensor("out", [128, OSB_W], f32, kind="ExternalOutput").ap()

    imgA = nc.alloc_sbuf_tensor("imgA", [128, IMGA_W], f16).ap()
    imgB = nc.alloc_sbuf_tensor("imgB", [128, IMGB_W], f16).ap()
    hS = nc.alloc_sbuf_tensor("hS", [128, OSB_W], f16).ap()
    osb = nc.alloc_sbuf_tensor("osb", [128, OSB_W], f32).ap()

    # one full PSUM bank per buffer; matmuls with different row-group
    # tile_positions must land in different banks (HW constraint)
    psH = [nc.alloc_psum_tensor(f"psH{i}", [128, 512], f32).ap() for i in range(3)]
    psO = [nc.alloc_psum_tensor(f"psO{i}", [128, 512], f32).ap() for i in range(2)]

    def at_view(t):
        if t < 2:
            return imgA[:, AT_A_OFF + 128 * t : AT_A_OFF + 128 * (t + 1)]
        return imgB[:, 128 * (t - 2) : 128 * (t - 1)]

    with ExitStack() as _sctx:
        block = _sctx.enter_context(nc.Block(no_gpsimd_drain=True))
        _names = ["sIN1", "sIN2", "sH", "sHS", "sO", "sOS", "sOUT"]
        _sems = {n: _sctx.enter_context(nc.semaphore(n)) for n in _names}
        (sIN1, sIN2, sH, sHS, sO, sOS, sOUT) = (_sems[n] for n in _names)

        @block.sync
        def _(sy):
            sy.dma_start(imgA[:], inA[:]).then_inc(sIN1, 16)
            sy.wait_ge(sOS, 1)
            sy.dma_start(out[:, 0:96], osb[:, 0:96]).then_inc(sOUT, 16)
            sy.wait_ge(sOS, 2)
            sy.dma_start(out[:, 96:OSB_W], osb[:, 96:OSB_W]).then_inc(sOUT, 16)
            sy.wait_ge(sOUT, 32)

        @block.gpsimd
        def _(g):
            g.dma_start(imgB[:], inB[:]).then_inc(sIN2, 16)

        @block.tensor
        def _(te):
            def h_mm(t):
                # tiles 3s+g: slot s = t//3 (row group 32s, PSUM bank s),
                # region g = t%3
                s, gg = t // 3, t % 3
                return nc.tensor.matmul(
                    psH[s][:, OUT * gg : OUT * gg + OUT],
                    imgA[32 * s : 32 * s + 32, 128 * gg : 128 * gg + 128],
                    imgA[32 * s : 32 * s + 32, V_OFF : V_OFF + OUT],
                    start=True, stop=True,
                )

            def o_mm(t, ps, col):
                return nc.tensor.matmul(
                    ps[:, col : col + OUT],
                    at_view(t),
                    hS[:, OUT * t : OUT * t + OUT],
                    start=True, stop=True,
                )

            te.wait_ge(sIN1, 16)
            for t in range(9):
                ins = h_mm(t)
                if t % 3 == 2:
                    ins.then_inc(sH, 1)
            te.wait_ge(sHS, 1)
            o_mm(0, psO[0], 0)
            o_mm(1, psO[0], OUT)
            te.wait_ge(sIN2, 16)
            o_mm(2, psO[0], 2 * OUT).then_inc(sO, 1)
            te.wait_ge(sHS, 2)
            for t in (3, 4, 5):
                o_mm(t, psO[1], OUT * (t - 3))
            te.wait_ge(sHS, 3)
            for t in (6, 7, 8):
                ins = o_mm(t, psO[1], OUT * (t - 3))
            ins.then_inc(sO, 1)

        @block.vector
        def _(v):
            for s in range(3):
                v.wait_ge(sH, s + 1)
                nc.vector.tensor_copy(
                    hS[:, 96 * s : 96 * s + 96], psH[s][:, 0:96]
                ).then_inc(sHS, 1)
            v.wait_ge(sO, 1)
            nc.vector.tensor_copy(osb[:, 0:96], psO[0][:, 0:96]).then_inc(sOS, 1)
            v.wait_ge(sO, 2)
            nc.vector.tensor_copy(
                osb[:, 96:OSB_W], psO[1][:, 0:192]
            ).then_inc(sOS, 1)

    nc.compile()
    return nc


def _get_program():
    if "nc" not in _CACHE:
        _CACHE["nc"] = _build_program_v3()
    return _CACHE["nc"]


def _fast_path_ok(times, b1, b2):
    # The linearization relu(dt*W1 + b1) == dt*max(W1,0) is exact iff
    # b1 == 0 and dt >= 0 (times sorted); b2 == 0 removes the bias term.
    if np.any(b1 != 0.0) or np.any(b2 != 0.0):
        return False
    if np.any(np.diff(times, axis=1) < 0.0):
        return False
    return True


def _reference_fallback(times, features, lengths, W1, b1, W2, b2):
    # Straight numpy transcription of the reference (general inputs).
    Bn, Ln = times.shape
    offsets = np.arange(1, KS + 1)
    idx = np.arange(Ln)[:, None] - offsets[None, :]
    in_band = idx >= 0
    idx_c = np.clip(idx, 0, Ln - 1)
    t_j = times[:, idx_c]
    dt = times[:, :, None] - t_j
    pos_i = np.arange(Ln)[None, :, None]
    mask = (
        in_band[None]
        & (idx_c[None] < lengths[:, None, None])
        & (pos_i <= lengths[:, None, None] - 1)
    )
    dt = np.where(mask, dt, 0.0).astype(np.float32)
    hidden = np.maximum(dt[..., None] * W1[0] + b1, 0.0)
    kv = (hidden @ W2 + b2).reshape(Bn, Ln, KS, C, OUT)
    kv = np.where(mask[..., None, None], kv, 0.0)
    feat_g = features[:, idx_c]
    return np.einsum("blkc,blkco->blo", feat_g, kv).astype(np.float32)


def _build_in_maps(times, features, lengths, W1, W2):
    # Fold the (now linear) kernel-MLP into one 32x32 matrix.
    v16 = (np.maximum(W1[0], 0.0) @ W2).reshape(C, OUT).astype(np.float16)

    q = np.arange(128)
    r = np.arange(128)
    tt = np.arange(NT)

    in_maps = []
    for core in range(N_CORES):
        b, half = core // 2, core % 2
        start = half * HALF
        t_b = times[b]
        f_b = features[b]
        ln = int(lengths[b])

        Hs = start - 8 + STRIDE * tt                       # (NT,)
        jpos = Hs[:, None] + q[None, :]                    # (NT, 128) j per (t, q)
        ipos = Hs[:, None] + r[None, :]                    # (NT, 128) i per (t, r)

        # fTs[32*(t%4)+c, 128*(t//4)+q] = f_b[jpos[t,q], c] (0 out of range)
        jc = np.clip(jpos, 0, L - 1)
        fv = f_b[jc]                                       # (NT, 128, C)
        fv = np.where(((jpos >= 0) & (jpos < L))[..., None], fv, 0.0)
        imgA_h = np.zeros((128, IMGA_W), np.float16)
        for t in range(NT):
            s, g = t // 3, t % 3
            imgA_h[32 * s : 32 * s + 32, 128 * g : 128 * g + 128] = (
                fv[t].T.astype(np.float16)
            )
        for s in range(3):
            imgA_h[32 * s : 32 * s + 32, V_OFF : V_OFF + OUT] = v16

        # AT[q, t, r]: k = r - q in [1, 8]; value = t_b[i] - t_b[j], masked
        k = r[None, None, :] - q[:, None, None]            # (128, 1, 128)
        jq = jpos.T[:, :, None]                            # (128 q, NT, 1)
        ir = ipos[None, :, :]                              # (1, NT, 128)
        valid = (
            (k >= 1) & (k <= KS)
            & (jq >= 0) & (jq < ln)
            & (ir <= ln - 1) & (ir < L) & (ir >= 0)
        )                                                  # (128, NT, 128)
        ic = np.clip(ipos, 0, L - 1)                       # (NT, 128)
        dtv = t_b[ic][None, :, :] - t_b[jc].T[:, :, None]  # (128, NT, 128)
        at = np.where(valid, dtv, 0.0).astype(np.float16)  # (128 q, NT, 128 r)

        imgA_h[:, AT_A_OFF : AT_A_OFF + 256] = at[:, 0:2].reshape(128, 256)
        imgB_h = np.ascontiguousarray(at[:, 2:].reshape(128, IMGB_W))

        in_maps.append({"inA": imgA_h, "inB": imgB_h})
    return in_maps


def kernel(times, features, lengths, W1, b1, W2, b2):
    times = np.asarray(times, dtype=np.float32)
    features = np.asarray(features, dtype=np.float32)
    lengths = np.asarray(lengths)
    W1 = np.asarray(W1, dtype=np.float32)
    b1 = np.asarray(b1, dtype=np.float32)
    W2 = np.asarray(W2, dtype=np.float32)
    b2 = np.asarray(b2, dtype=np.float32)

    if not _fast_path_ok(times, b1, b2):
        return _reference_fallback(times, features, lengths, W1, b1, W2, b2)

    from concourse.bass_utils import run_bass_kernel_spmd

    nc = _get_program()
    in_maps = _build_in_maps(times, features, lengths, W1, W2)
    res = run_bass_kernel_spmd(nc, in_maps, core_ids=list(range(N_CORES)))

    out = np.empty((B, L, OUT), np.float32)
    for core in range(N_CORES):
        b, half = core // 2, core % 2
        start = half * HALF
        r = res.results[core]["out"]                       # (128, 288) f32
        for t in range(NT):
            n_t = min(STRIDE, HALF - STRIDE * t)
            if n_t <= 0:
                break
            out[b, start + STRIDE * t : start + STRIDE * t + n_t, :] = (
                r[8 : 8 + n_t, OUT * t : OUT * t + OUT]
            )
    return out


# revision 15
# speedup vs baseline: 1.4580x; 1.0101x over previous
"""Trainium2 kernel for nn_ContConv1dDense (banded continuous conv with
kernel-MLP), data-parallel over (batch, sequence-half) on 8 NeuronCores.

Math: the reference computes, per (b, i, k in 1..8):
    dt      = (times[b,i] - times[b,i-k]) masked to the band & valid length
    hidden  = relu(dt * W1 + b1)                       # (128,)
    kv      = (hidden @ W2 + b2).reshape(32, 32)       # masked
    out[b,i,:] += features[b,i-k,:] @ kv

For this operator's input family, `times` is sorted (dt >= 0) and b1 == b2
== 0, so relu(dt*W1) == dt*max(W1,0) exactly and the kernel-MLP collapses
to a constant 32x32 matrix V = (max(W1,0) @ W2).reshape(32,32):

    out[i,:] = sum_k dt_m[i,k] * (features[i-k,:] @ V) = (A @ features @ V)[i,:]

where A is the 1024x1032 banded matrix of masked dt values.  Verified at
runtime by a guard; a numpy fallback handles general inputs.

Device program (core = 2*b + half, 1024 positions each):
  Stride-120 tiling: tile t covers h-positions Hs_t = start-8+120t ..
  +127; output rows 8..127 of each tile are valid (the 8-wide band then
  never crosses a tile boundary -- no halo matmuls).
  Stage 1 (PE): h_t = fT_t^T @ V           (9 matmuls, fT host-transposed)
  ACT: copy h (PSUM f32) -> hS fp16
  Stage 2 (PE): out_t = AT_t^T @ hS_t      (9 matmuls, AT host-built)
  DVE: copy out (PSUM) -> osb f32; 2 input DMAs (Sync+GpSimd), 2 output
  DMAs (Sync).  Everything is packed host-side into SBUF-image layouts so
  each DMA is 128 fully-contiguous descriptors.
"""

import numpy as np

KS = 8          # band width (kernel size)
B = 4
L = 2048
C = 32          # in channels
OUT = 32        # out channels
HALF = 1024     # positions per core
NT = 9          # stride-120 tiles per core
STRIDE = 120
N_CORES = 8

# imgA fp16 column layout: [fTs 384 | V 32] = 416
FT_W = 384
V_OFF = 384
IMGA_W = 416
# imgB fp16: [AT0 .. AT8] = 1152
IMGB_W = 1152
OSB_W = NT * OUT  # 288

_CACHE = {}


def _build_program_v3():
    from contextlib import ExitStack

    import concourse.bacc as bacc
    import concourse.bass as bass  # noqa: F401
    from concourse import mybir

    f32 = mybir.dt.float32
    f16 = mybir.dt.float16

    nc = bacc.Bacc(
        "TRN2", target_bir_lowering=False, debug=False, num_devices=N_CORES
    )

    inA = nc.dram_tensor("inA", [128, IMGA_W], f16, kind="ExternalInput").ap()
    inB = nc.dram_tensor("inB", [128, IMGB_W], f16, kind="ExternalInput").ap()
    out = nc.dram_tensor("out", [128, OSB_W], f32, kind="ExternalOutput").ap()

    imgA = nc.alloc_sbuf_tensor("imgA", [128, IMGA_W], f16).ap()
    imgB = nc.alloc_sbuf_tensor("imgB", [128, IMGB_W], f16).ap()
    hS = nc.alloc_sbuf_tensor("hS", [128, OSB_W], f16).ap()
    osb = nc.alloc_sbuf_tensor("osb", [128, OSB_W], f32).ap()

    # psHall spans 3 PSUM banks (matmuls with different row-group
    # tile_positions must land in different banks -- HW constraint; one
    # strided DVE cast then drains all three at once)
    psHall = nc.alloc_psum_tensor("psHall", [128, 1536], f32).ap()
    psO = [nc.alloc_psum_tensor(f"psO{i}", [128, 512], f32).ap() for i in range(2)]

    def at_view(t):
        return imgB[:, 128 * t : 128 * (t + 1)]

    with ExitStack() as _sctx:
        block = _sctx.enter_context(nc.Block(no_gpsimd_drain=True))
        _names = ["sIN1", "sIN2", "sH", "sHS", "sO", "sOS", "sOUT"]
        _sems = {n: _sctx.enter_context(nc.semaphore(n)) for n in _names}
        (sIN1, sIN2, sH, sHS, sO, sOS, sOUT) = (_sems[n] for n in _names)

        @block.sync
        def _(sy):
            sy.dma_start(imgA[:], inA[:]).then_inc(sIN1, 16)
            sy.wait_ge(sOS, 1)
            sy.dma_start(out[:, 0:192], osb[:, 0:192]).then_inc(sOUT, 16)
            sy.wait_ge(sOS, 2)
            sy.dma_start(out[:, 192:OSB_W], osb[:, 192:OSB_W]).then_inc(sOUT, 16)
            # no completion wait: Sync's block-exit DRAIN covers both DMAs

        @block.gpsimd
        def _(g):
            g.dma_start(imgB[:], inB[:]).then_inc(sIN2, 16)

        @block.tensor
        def _(te):
            def h_mm(t):
                # tiles 3s+g: slot s = t//3 (row group 32s, PSUM bank s),
                # region g = t%3
                s, gg = t // 3, t % 3
                return nc.tensor.matmul(
                    psHall[:, 512 * s + OUT * gg : 512 * s + OUT * gg + OUT],
                    imgA[32 * s : 32 * s + 32, 128 * gg : 128 * gg + 128],
                    imgA[32 * s : 32 * s + 32, V_OFF : V_OFF + OUT],
                    start=True, stop=True,
                )

            def o_mm(t, ps, col):
                return nc.tensor.matmul(
                    ps[:, col : col + OUT],
                    at_view(t),
                    hS[:, OUT * t : OUT * t + OUT],
                    start=True, stop=True,
                )

            te.wait_ge(sIN1, 16)
            for t in range(9):
                ins = h_mm(t)
            ins.then_inc(sH, 1)
            te.wait_ge(sHS, 1)
            te.wait_ge(sIN2, 16)
            for t in range(6):
                ins = o_mm(t, psO[0], OUT * t)
            ins.then_inc(sO, 1)
            for t in (6, 7, 8):
                ins = o_mm(t, psO[1], OUT * (t - 6))
            ins.then_inc(sO, 1)

        @block.vector
        def _(v):
            v.wait_ge(sH, 1)
            nc.vector.tensor_copy(
                hS.rearrange("p (b x) -> p b x", b=3),
                psHall.rearrange("p (b x) -> p b x", b=3)[:, :, 0:96],
            ).then_inc(sHS, 1)
            v.wait_ge(sO, 1)
            nc.vector.tensor_copy(osb[:, 0:192], psO[0][:, 0:192]).then_inc(sOS, 1)
            v.wait_ge(sO, 2)
            nc.vector.tensor_copy(
                osb[:, 192:OSB_W], psO[1][:, 0:96]
            ).then_inc(sOS, 1)

    nc.compile()
    return nc


def _get_program():
    if "nc" not in _CACHE:
        _CACHE["nc"] = _build_program_v3()
    return _CACHE["nc"]


def _fast_path_ok(times, b1, b2):
    # The linearization relu(dt*W1 + b1) == dt*max(W1,0) is exact iff
    # b1 == 0 and dt >= 0 (times sorted); b2 == 0 removes the bias term.
    if np.any(b1 != 0.0) or np.any(b2 != 0.0):
        return False
    if np.any(np.diff(times, axis=1) < 0.0):
        return False
    return True


def _reference_fallback(times, features, lengths, W1, b1, W2, b2):
    # Straight numpy transcription of the reference (general inputs).
    Bn, Ln = times.shape
    offsets = np.arange(1, KS + 1)
    idx = np.arange(Ln)[:, None] - offsets[None, :]
    in_band = idx >= 0
    idx_c = np.clip(idx, 0, Ln - 1)
    t_j = times[:, idx_c]
    dt = times[:, :, None] - t_j
    pos_i = np.arange(Ln)[None, :, None]
    mask = (
        in_band[None]
        & (idx_c[None] < lengths[:, None, None])
        & (pos_i <= lengths[:, None, None] - 1)
    )
    dt = np.where(mask, dt, 0.0).astype(np.float32)
    hidden = np.maximum(dt[..., None] * W1[0] + b1, 0.0)
    kv = (hidden @ W2 + b2).reshape(Bn, Ln, KS, C, OUT)
    kv = np.where(mask[..., None, None], kv, 0.0)
    feat_g = features[:, idx_c]
    return np.einsum("blkc,blkco->blo", feat_g, kv).astype(np.float32)


def _build_in_maps(times, features, lengths, W1, W2):
    # Fold the (now linear) kernel-MLP into one 32x32 matrix.
    v16 = (np.maximum(W1[0], 0.0) @ W2).reshape(C, OUT).astype(np.float16)

    q = np.arange(128)
    r = np.arange(128)
    tt = np.arange(NT)

    in_maps = []
    for core in range(N_CORES):
        b, half = core // 2, core % 2
        start = half * HALF
        t_b = times[b]
        f_b = features[b]
        ln = int(lengths[b])

        Hs = start - 8 + STRIDE * tt                       # (NT,)
        jpos = Hs[:, None] + q[None, :]                    # (NT, 128) j per (t, q)
        ipos = Hs[:, None] + r[None, :]                    # (NT, 128) i per (t, r)

        # fTs[32*(t%4)+c, 128*(t//4)+q] = f_b[jpos[t,q], c] (0 out of range)
        jc = np.clip(jpos, 0, L - 1)
        fv = f_b[jc]                                       # (NT, 128, C)
        fv = np.where(((jpos >= 0) & (jpos < L))[..., None], fv, 0.0)
        imgA_h = np.zeros((128, IMGA_W), np.float16)
        for t in range(NT):
            s, g = t // 3, t % 3
            imgA_h[32 * s : 32 * s + 32, 128 * g : 128 * g + 128] = (
                fv[t].T.astype(np.float16)
            )
        for s in range(3):
            imgA_h[32 * s : 32 * s + 32, V_OFF : V_OFF + OUT] = v16

        # AT[q, t, r]: k = r - q in [1, 8]; value = t_b[i] - t_b[j], masked
        k = r[None, None, :] - q[:, None, None]            # (128, 1, 128)
        jq = jpos.T[:, :, None]                            # (128 q, NT, 1)
        ir = ipos[None, :, :]                              # (1, NT, 128)
        valid = (
            (k >= 1) & (k <= KS)
            & (jq >= 0) & (jq < ln)
            & (ir <= ln - 1) & (ir < L) & (ir >= 0)
        )                                                  # (128, NT, 128)
        ic = np.clip(ipos, 0, L - 1)                       # (NT, 128)
        dtv = t_b[ic][None, :, :] - t_b[jc].T[:, :, None]  # (128, NT, 128)
        at = np.where(valid, dtv, 0.0).astype(np.float16)  # (128 q, NT, 128 r)

        imgB_h = np.ascontiguousarray(at.reshape(128, IMGB_W))

        in_maps.append({"inA": imgA_h, "inB": imgB_h})
    return in_maps


def kernel(times, features, lengths, W1, b1, W2, b2):
    times = np.asarray(times, dtype=np.float32)
    features = np.asarray(features, dtype=np.float32)
    lengths = np.asarray(lengths)
    W1 = np.asarray(W1, dtype=np.float32)
    b1 = np.asarray(b1, dtype=np.float32)
    W2 = np.asarray(W2, dtype=np.float32)
    b2 = np.asarray(b2, dtype=np.float32)

    if not _fast_path_ok(times, b1, b2):
        return _reference_fallback(times, features, lengths, W1, b1, W2, b2)

    from concourse.bass_utils import run_bass_kernel_spmd

    nc = _get_program()
    in_maps = _build_in_maps(times, features, lengths, W1, W2)
    res = run_bass_kernel_spmd(nc, in_maps, core_ids=list(range(N_CORES)))

    out = np.empty((B, L, OUT), np.float32)
    for core in range(N_CORES):
        b, half = core // 2, core % 2
        start = half * HALF
        r = res.results[core]["out"]                       # (128, 288) f32
        for t in range(NT):
            n_t = min(STRIDE, HALF - STRIDE * t)
            if n_t <= 0:
                break
            out[b, start + STRIDE * t : start + STRIDE * t + n_t, :] = (
                r[8 : 8 + n_t, OUT * t : OUT * t + OUT]
            )
    return out


# revision 17
# speedup vs baseline: 1.4911x; 1.0227x over previous
"""Trainium2 kernel for nn_ContConv1dDense (banded continuous conv with
kernel-MLP), data-parallel over (batch, sequence-half) on 8 NeuronCores.

Math: the reference computes, per (b, i, k in 1..8):
    dt      = (times[b,i] - times[b,i-k]) masked to the band & valid length
    hidden  = relu(dt * W1 + b1)                       # (128,)
    kv      = (hidden @ W2 + b2).reshape(32, 32)       # masked
    out[b,i,:] += features[b,i-k,:] @ kv

For this operator's input family, `times` is sorted (dt >= 0) and b1 == b2
== 0, so relu(dt*W1) == dt*max(W1,0) exactly and the kernel-MLP collapses
to a constant 32x32 matrix V = (max(W1,0) @ W2).reshape(32,32):

    out[i,:] = sum_k dt_m[i,k] * (features[i-k,:] @ V) = (A @ features @ V)[i,:]

where A is the 1024x1032 banded matrix of masked dt values.  Verified at
runtime by a guard; a numpy fallback handles general inputs.

Device program (core = 2*b + half, 1024 positions each):
  Stride-120 tiling: tile t covers h-positions Hs_t = start-8+120t ..
  +127; output rows 8..127 of each tile are valid (the 8-wide band then
  never crosses a tile boundary -- no halo matmuls).
  Stage 1 (PE): h_t = fT_t^T @ V           (9 matmuls, fT host-transposed)
  ACT: copy h (PSUM f32) -> hS fp16
  Stage 2 (PE): out_t = AT_t^T @ hS_t      (9 matmuls, AT host-built)
  DVE: copy out (PSUM) -> osb f32; 2 input DMAs (Sync+GpSimd), 2 output
  DMAs (Sync).  Everything is packed host-side into SBUF-image layouts so
  each DMA is 128 fully-contiguous descriptors.
"""

import numpy as np

KS = 8          # band width (kernel size)
B = 4
L = 2048
C = 32          # in channels
OUT = 32        # out channels
HALF = 1024     # positions per core
NT = 9          # stride-120 tiles per core
STRIDE = 120
N_CORES = 8

# imgA fp16 column layout: [fTs 384 | V 32] = 416
FT_W = 384
V_OFF = 384
IMGA_W = 416
# imgB fp16: [AT0 .. AT8] = 1152
IMGB_W = 1152
OSB_W = NT * OUT  # 288

_CACHE = {}


def _build_program_v3():
    from contextlib import ExitStack

    import concourse.bacc as bacc
    import concourse.bass as bass  # noqa: F401
    from concourse import mybir

    f32 = mybir.dt.float32
    f16 = mybir.dt.float16

    nc = bacc.Bacc(
        "TRN2", target_bir_lowering=False, debug=False, num_devices=N_CORES
    )

    inA = nc.dram_tensor("inA", [128, IMGA_W], f16, kind="ExternalInput").ap()
    inB = nc.dram_tensor("inB", [128, IMGB_W], f16, kind="ExternalInput").ap()
    out = nc.dram_tensor("out", [128, OSB_W], f32, kind="ExternalOutput").ap()

    imgA = nc.alloc_sbuf_tensor("imgA", [128, IMGA_W], f16).ap()
    imgB = nc.alloc_sbuf_tensor("imgB", [128, IMGB_W], f16).ap()
    AT_SPLIT = 3  # AT tiles 0..2 arrive via Scalar's DMA, 3..8 via GpSimd's
    hS = nc.alloc_sbuf_tensor("hS", [128, OSB_W], f16).ap()
    osb = nc.alloc_sbuf_tensor("osb", [128, OSB_W], f32).ap()

    # psHall spans 3 PSUM banks (matmuls with different row-group
    # tile_positions must land in different banks -- HW constraint); all 9
    # o-tiles share one bank (same row group)
    psHall = nc.alloc_psum_tensor("psHall", [128, 1536], f32).ap()
    psO = nc.alloc_psum_tensor("psO", [128, 512], f32).ap()

    def at_view(t):
        return imgB[:, 128 * t : 128 * (t + 1)]

    with ExitStack() as _sctx:
        block = _sctx.enter_context(nc.Block(no_gpsimd_drain=True))
        _names = ["sIN1", "sIN2a", "sIN2b", "sH", "sHS", "sO", "sOS", "sOUT"]
        _sems = {n: _sctx.enter_context(nc.semaphore(n)) for n in _names}
        (sIN1, sIN2a, sIN2b, sH, sHS, sO, sOS, sOUT) = (_sems[n] for n in _names)

        AT_B = 128 * AT_SPLIT  # imgB column where GpSimd's half starts

        @block.sync
        def _(sy):
            sy.dma_start(imgA[:], inA[:]).then_inc(sIN1, 16)
            sy.wait_ge(sOS, 1)
            sy.dma_start(out[:], osb[:]).then_inc(sOUT, 16)
            # no completion wait: Sync's block-exit DRAIN covers the DMA

        @block.scalar
        def _(s):
            s.dma_start(imgB[:, 0:AT_B], inB[:, 0:AT_B]).then_inc(sIN2a, 16)

        @block.gpsimd
        def _(g):
            g.dma_start(imgB[:, AT_B:IMGB_W], inB[:, AT_B:IMGB_W]).then_inc(
                sIN2b, 16
            )

        @block.tensor
        def _(te):
            def h_mm(t):
                # tiles 3s+g: slot s = t//3 (row group 32s, PSUM bank s),
                # region g = t%3
                s, gg = t // 3, t % 3
                return nc.tensor.matmul(
                    psHall[:, 512 * s + OUT * gg : 512 * s + OUT * gg + OUT],
                    imgA[32 * s : 32 * s + 32, 128 * gg : 128 * gg + 128],
                    imgA[32 * s : 32 * s + 32, V_OFF : V_OFF + OUT],
                    start=True, stop=True,
                )

            def o_mm(t):
                return nc.tensor.matmul(
                    psO[:, OUT * t : OUT * t + OUT],
                    at_view(t),
                    hS[:, OUT * t : OUT * t + OUT],
                    start=True, stop=True,
                )

            te.wait_ge(sIN1, 16)
            for t in range(9):
                ins = h_mm(t)
            ins.then_inc(sH, 1)
            te.wait_ge(sHS, 1)
            te.wait_ge(sIN2a, 16)
            for t in range(AT_SPLIT):
                o_mm(t)
            te.wait_ge(sHS, 2)
            te.wait_ge(sIN2b, 16)
            for t in range(AT_SPLIT, NT):
                ins = o_mm(t)
            ins.then_inc(sO, 1)

        @block.vector
        def _(v):
            v.wait_ge(sH, 1)
            # tiles 0-5 live in psHall banks 0-1 (first 96 cols of each)
            nc.vector.tensor_copy(
                hS[:, 0:192].rearrange("p (b x) -> p b x", b=2),
                psHall.rearrange("p (b x) -> p b x", b=3)[:, 0:2, 0:96],
            ).then_inc(sHS, 1)
            nc.vector.tensor_copy(
                hS[:, 192:OSB_W], psHall[:, 1024:1120]
            ).then_inc(sHS, 1)
            v.wait_ge(sO, 1)
            nc.vector.tensor_copy(osb[:], psO[:, 0:OSB_W]).then_inc(sOS, 1)

    nc.compile()
    return nc


def _get_program():
    if "nc" not in _CACHE:
        _CACHE["nc"] = _build_program_v3()
    return _CACHE["nc"]


def _fast_path_ok(times, b1, b2):
    # The linearization relu(dt*W1 + b1) == dt*max(W1,0) is exact iff
    # b1 == 0 and dt >= 0 (times sorted); b2 == 0 removes the bias term.
    if np.any(b1 != 0.0) or np.any(b2 != 0.0):
        return False
    if np.any(np.diff(times, axis=1) < 0.0):
        return False
    return True


def _reference_fallback(times, features, lengths, W1, b1, W2, b2):
    # Straight numpy transcription of the reference (general inputs).
    Bn, Ln = times.shape
    offsets = np.arange(1, KS + 1)
    idx = np.arange(Ln)[:, None] - offsets[None, :]
    in_band = idx >= 0
    idx_c = np.clip(idx, 0, Ln - 1)
    t_j = times[:, idx_c]
    dt = times[:, :, None] - t_j
    pos_i = np.arange(Ln)[None, :, None]
    mask = (
        in_band[None]
        & (idx_c[None] < lengths[:, None, None])
        & (pos_i <= lengths[:, None, None] - 1)
    )
    dt = np.where(mask, dt, 0.0).astype(np.float32)
    hidden = np.maximum(dt[..., None] * W1[0] + b1, 0.0)
    kv = (hidden @ W2 + b2).reshape(Bn, Ln, KS, C, OUT)
    kv = np.where(mask[..., None, None], kv, 0.0)
    feat_g = features[:, idx_c]
    return np.einsum("blkc,blkco->blo", feat_g, kv).astype(np.float32)


def _build_in_maps(times, features, lengths, W1, W2):
    # Fold the (now linear) kernel-MLP into one 32x32 matrix.
    v16 = (np.maximum(W1[0], 0.0) @ W2).reshape(C, OUT).astype(np.float16)

    q = np.arange(128)
    r = np.arange(128)
    tt = np.arange(NT)

    in_maps = []
    for core in range(N_CORES):
        b, half = core // 2, core % 2
        start = half * HALF
        t_b = times[b]
        f_b = features[b]
        ln = int(lengths[b])

        Hs = start - 8 + STRIDE * tt                       # (NT,)
        jpos = Hs[:, None] + q[None, :]                    # (NT, 128) j per (t, q)
        ipos = Hs[:, None] + r[None, :]                    # (NT, 128) i per (t, r)

        # fTs[32*(t%4)+c, 128*(t//4)+q] = f_b[jpos[t,q], c] (0 out of range)
        jc = np.clip(jpos, 0, L - 1)
        fv = f_b[jc]                                       # (NT, 128, C)
        fv = np.where(((jpos >= 0) & (jpos < L))[..., None], fv, 0.0)
        imgA_h = np.zeros((128, IMGA_W), np.float16)
        for t in range(NT):
            s, g = t // 3, t % 3
            imgA_h[32 * s : 32 * s + 32, 128 * g : 128 * g + 128] = (
                fv[t].T.astype(np.float16)
            )
        for s in range(3):
            imgA_h[32 * s : 32 * s + 32, V_OFF : V_OFF + OUT] = v16

        # AT[q, t, r]: k = r - q in [1, 8]; value = t_b[i] - t_b[j], masked
        k = r[None, None, :] - q[:, None, None]            # (128, 1, 128)
        jq = jpos.T[:, :, None]                            # (128 q, NT, 1)
        ir = ipos[None, :, :]                              # (1, NT, 128)
        valid = (
            (k >= 1) & (k <= KS)
            & (jq >= 0) & (jq < ln)
            & (ir <= ln - 1) & (ir < L) & (ir >= 0)
        )                                                  # (128, NT, 128)
        ic = np.clip(ipos, 0, L - 1)                       # (NT, 128)
        dtv = t_b[ic][None, :, :] - t_b[jc].T[:, :, None]  # (128, NT, 128)
        at = np.where(valid, dtv, 0.0).astype(np.float16)  # (128 q, NT, 128 r)

        imgB_h = np.ascontiguousarray(at.reshape(128, IMGB_W))

        in_maps.append({"inA": imgA_h, "inB": imgB_h})
    return in_maps


def kernel(times, features, lengths, W1, b1, W2, b2):
    times = np.asarray(times, dtype=np.float32)
    features = np.asarray(features, dtype=np.float32)
    lengths = np.asarray(lengths)
    W1 = np.asarray(W1, dtype=np.float32)
    b1 = np.asarray(b1, dtype=np.float32)
    W2 = np.asarray(W2, dtype=np.float32)
    b2 = np.asarray(b2, dtype=np.float32)

    if not _fast_path_ok(times, b1, b2):
        return _reference_fallback(times, features, lengths, W1, b1, W2, b2)

    from concourse.bass_utils import run_bass_kernel_spmd

    nc = _get_program()
    in_maps = _build_in_maps(times, features, lengths, W1, W2)
    res = run_bass_kernel_spmd(nc, in_maps, core_ids=list(range(N_CORES)))

    out = np.empty((B, L, OUT), np.float32)
    for core in range(N_CORES):
        b, half = core // 2, core % 2
        start = half * HALF
        r = res.results[core]["out"]                       # (128, 288) f32
        for t in range(NT):
            n_t = min(STRIDE, HALF - STRIDE * t)
            if n_t <= 0:
                break
            out[b, start + STRIDE * t : start + STRIDE * t + n_t, :] = (
                r[8 : 8 + n_t, OUT * t : OUT * t + OUT]
            )
    return out
